# revision 1
# baseline (speedup 1.0000x reference)
"""Trainium2 Bass kernel for the two-level softmax-pooled text/video retrieval head.

Computes, for text_feat [256,32,512], video_feat [256,16,512], text_mask [256,32]:
    out[a,b] = (t2v(a,b) + v2t(a,b)) / 2
where t2v/v2t are two-level softmax-weighted poolings of the cross token/frame
cosine similarity tensor logits[a,b,t,v] (see reference module).

Sharding: text axis A split across 8 NeuronCores (32 queries each); video
features replicated. Host does l2-normalization + transposition (layout prep);
the device does all einsum + softmax compute.

Device algorithm per core (A_loc=32, T=32, B=256, V=16, D=512):
  - logits tiles [128=(q,t), 512=(b,v)] = tT.T @ vT  (fp32r matmuls, K=512)
  - E = exp(TAU*logits - 30)  (ACT; global -30 shift is softmax-invariant and
    keeps everything in fp32 normal range for |cos| < 0.58)
  - XE = X*E on DVE (measured faster on HW than a GPSIMD offload, which
    loses to SBUF-port contention and the extra PSUM->SBUF copy)
  - t2v path: per-row v-groups of 16: S1=sum(E), N1=sum(XE) DVE reduces;
    t2v = N1/S1; second level over t via mask-valued selector matmuls on PE.
  - v2t path: sums over t (partition axis) via mask-valued selector matmuls
    (Den2 = sel.T @ E, Num2 = sel.T @ XE) accumulated in PSUM; second level
    over v via DVE group reduces.
  - text_mask handling is exact: mask values (0/1) live in the selector
    matrices, so padded tokens contribute exactly 0 to every over-t sum.
"""

import os
import sys

import numpy as np

if "/opt/trn_rl_repo" not in sys.path:
    sys.path.insert(0, "/opt/trn_rl_repo")

A, T_TOK, B, V_FRM, D = 256, 32, 256, 16, 512
N_CORES = 8
A_LOC = A // N_CORES            # 32 queries per core
M_ROWS = A_LOC * T_TOK          # 1024  (q,t) rows
N_COLS = B * V_FRM              # 4096  (b,v) cols
N_MT = M_ROWS // 128            # 8 M-tiles (4 queries each)
N_NT = N_COLS // 512            # 8 N-tiles (32 videos each)
N_KC = D // 128                 # 4 K-chunks
TAU = 100.0
SHIFT = -30.0                   # global exp shift (softmax-invariant)
EPS = 1e-6

_PROGRAM_CACHE = {}


def _build_program(reps=1, xe_pool_eng=False):
    import contextlib

    import concourse.mybir as mybir
    import concourse.tile as tile
    from concourse import bacc

    f32 = mybir.dt.float32
    f32r = mybir.dt.float32r
    EXP = mybir.ActivationFunctionType.Exp
    MUL = mybir.AluOpType.mult
    ADD = mybir.AluOpType.add
    AX = mybir.AxisListType.X

    nc = bacc.Bacc("TRN2", target_bir_lowering=False, debug=False)

    tT_d = nc.dram_tensor("tT", [D, M_ROWS], f32r, kind="ExternalInput")
    vT_d = nc.dram_tensor("vT", [D, N_COLS], f32r, kind="ExternalInput")
    sel_d = nc.dram_tensor("sel", [128, N_MT * 32], f32r, kind="ExternalInput")
    sele_d = nc.dram_tensor("sele", [128, N_MT * 224], f32r, kind="ExternalInput")
    # bias width varies with reps/eng so each build gets a distinct HLO hash
    # (the NEFF cache otherwise silently reuses the first-compiled program)
    bias_cols = N_MT + 1 + (reps - 1) + (7 if xe_pool_eng else 0)
    bias_d = nc.dram_tensor("bias", [128, bias_cols], f32, kind="ExternalInput")
    out_d = nc.dram_tensor("out", [A_LOC, B], f32, kind="ExternalOutput")

    with tile.TileContext(nc) as tc, contextlib.ExitStack() as ctx:
        persist = ctx.enter_context(tc.tile_pool(name="persist", bufs=1))
        ps_pool = ctx.enter_context(tc.tile_pool(name="ps", bufs=4, space="PSUM"))
        dn2_pool = ctx.enter_context(tc.tile_pool(name="dn2", bufs=1, space="PSUM"))
        dn3_pool = ctx.enter_context(tc.tile_pool(name="dn3", bufs=1, space="PSUM"))
        e_pool = ctx.enter_context(tc.tile_pool(name="e", bufs=10))
        xs_pool = ctx.enter_context(tc.tile_pool(name="xs", bufs=4))
        t2v_pool = ctx.enter_context(tc.tile_pool(name="t2v", bufs=2))
        w_pool = ctx.enter_context(tc.tile_pool(name="w", bufs=3))
        v_pool = ctx.enter_context(tc.tile_pool(name="v2", bufs=2))

        # ---- persistent inputs (emission order == DMA priority: text and
        # selectors first, then video in n-major order so early N-tiles land
        # before late ones) ----
        tt_tiles = []
        for k in range(N_KC):
            t_ = persist.tile([128, M_ROWS], f32r, tag=f"tt_{k}")
            nc.sync.dma_start(out=t_[:], in_=tT_d.ap()[128 * k:128 * (k + 1), :])
            tt_tiles.append(t_)
        sel_sb = persist.tile([128, N_MT * 32], f32r, tag="sel")
        nc.sync.dma_start(out=sel_sb[:], in_=sel_d.ap())
        sele_sb = persist.tile([128, N_MT * 224], f32r, tag="sele")
        nc.sync.dma_start(out=sele_sb[:], in_=sele_d.ap())
        bias_sb = persist.tile([128, bias_cols], f32, tag="bias")
        nc.sync.dma_start(out=bias_sb[:], in_=bias_d.ap())
        vt_tiles = {}
        for n in range(N_NT):
            for k in range(N_KC):
                t_ = persist.tile([128, 512], f32r, tag=f"vt_{k}_{n}")
                nc.sync.dma_start(
                    out=t_[:],
                    in_=vT_d.ap()[128 * k:128 * (k + 1), 512 * n:512 * (n + 1)],
                )
                vt_tiles[(k, n)] = t_

        # combined accumulator, side-major: col = side*2048 + m*256 + n*32 + b
        # (side 0=S1, 1=N1; side-major keeps phase-2 reads contiguous)
        sn_all = persist.tile([128, N_MT * 512], f32, tag="sn_all")
        vt2_stage = persist.tile([A_LOC, B], f32, tag="vt2_stage")

        for _rep in range(reps):
            # ---- main loop: halves (b 0:128 / 128:256) x N-tiles x M-tiles
            for h in range(2):
                den2 = dn2_pool.tile([128, 512], f32, tag="den2")
                num2 = dn2_pool.tile([128, 512], f32, tag="num2")
                for j in range(4):
                    n = 4 * h + j
                    first = (j == 0)
                    last = (j == 3)
                    exe_wave = []
                    for m in range(N_MT):
                        ps = ps_pool.tile([128, 512], f32, tag="ps")
                        for k in range(N_KC):
                            nc.tensor.matmul(
                                ps[:],
                                tt_tiles[k][:, 128 * m:128 * (m + 1)],
                                vt_tiles[(k, n)][:],
                                start=(k == 0),
                                stop=(k == N_KC - 1),
                            )
                        # E and XE live in one [128,1024] tile so both group
                        # reductions merge into a single DVE instruction
                        exe = e_pool.tile([128, 1024], f32r, tag="e")
                        nc.scalar.activation(
                            exe[:, 0:512], ps[:], EXP,
                            bias=bias_sb[:, m:m + 1], scale=TAU,
                        )
                        if xe_pool_eng:
                            xs_t = xs_pool.tile([128, 512], f32, tag="xs")
                            nc.scalar.copy(xs_t[:], ps[:])
                            nc.gpsimd.tensor_tensor(
                                exe[:, 512:1024], xs_t[:],
                                exe[:, 0:512].bitcast(f32), op=MUL)
                        else:
                            nc.vector.tensor_tensor(
                                exe[:, 512:1024], ps[:],
                                exe[:, 0:512].bitcast(f32), op=MUL)
                        exe_wave.append(exe)
                        # wave 1: E selector matmul
                        selw = sele_sb[:, m * 224 + 96 - 32 * j:
                                       m * 224 + 224 - 32 * j]
                        nc.tensor.matmul(
                            den2[:], selw, exe[:, 0:512],
                            start=(first and m == 0),
                            stop=(last and m == N_MT - 1),
                            skip_group_check=True,
                        )
                    # wave 2: XE selector matmuls + merged S1|N1 reductions
                    for m in range(N_MT):
                        exe = exe_wave[m]
                        selw = sele_sb[:, m * 224 + 96 - 32 * j:
                                       m * 224 + 224 - 32 * j]
                        nc.tensor.matmul(
                            num2[:], selw, exe[:, 512:1024],
                            start=(first and m == 0),
                            stop=(last and m == N_MT - 1),
                            skip_group_check=True,
                        )
                        col = m * 256 + n * 32
                        nc.vector.reduce_sum(
                            out=sn_all[:].rearrange(
                                "p (s mb) -> p s mb", s=2)[:, :, col:col + 32],
                            in_=exe[:].bitcast(f32).rearrange(
                                "p (s b v) -> p s b v", s=2, v=16),
                            axis=AX,
                        )
                # ---- second level of v2t for this half (softmax over v) ----
                den2_sb = v_pool.tile([128, 512], f32, tag="den2_sb")
                nc.scalar.copy(den2_sb[:], den2[:])
                rden2 = v_pool.tile([128, 512], f32, tag="rden2")
                nc.vector.reciprocal(rden2[:], den2_sb[:])
                v_t = v_pool.tile([128, 512], f32, tag="v_t")
                nc.vector.tensor_tensor(v_t[:], num2[:], rden2[:], op=MUL)
                exev = v_pool.tile([128, 1024], f32, tag="exev")
                nc.scalar.activation(
                    exev[:, 0:512], v_t[:], EXP, bias=bias_sb[:, N_MT:N_MT + 1],
                    scale=TAU)
                nc.vector.tensor_tensor(
                    exev[:, 512:1024], v_t[:], exev[:, 0:512], op=MUL)
                snv_t = v_pool.tile([128, 64], f32, tag="snv_t")
                nc.vector.reduce_sum(
                    out=snv_t[:],
                    in_=exev[:].rearrange("p (s b v) -> p s b v", s=2, v=16),
                    axis=AX)
                rsv_t = v_pool.tile([128, 32], f32, tag="rsv_t")
                nc.vector.reciprocal(rsv_t[:], snv_t[:, 0:32])
                v2t2 = v_pool.tile([128, 32], f32, tag="v2t2")
                # v2t2 = 0.5 * Nv / Sv   (the final /2 folded in here)
                nc.vector.scalar_tensor_tensor(
                    out=v2t2[:], in0=snv_t[:, 32:64], scalar=0.5, in1=rsv_t[:],
                    op0=MUL, op1=MUL,
                )
                for j in range(4):
                    nc.sync.dma_start(
                        out=vt2_stage[0:32,
                                      128 * h + 32 * j:128 * h + 32 * (j + 1)],
                        in_=v2t2[32 * j:32 * (j + 1), :],
                    )

            # ---- second level of t2v (softmax over t via selector MMs) ----
            den3 = dn3_pool.tile([32, 256], f32, tag="den3")
            num3 = dn3_pool.tile([32, 256], f32, tag="num3")
            for m in range(N_MT):
                mb = m * 256
                rs1 = t2v_pool.tile([128, 256], f32, tag="rs1")
                nc.vector.reciprocal(rs1[:], sn_all[:, mb:mb + 256])
                t2v_t = t2v_pool.tile([128, 256], f32, tag="t2v_t")
                nc.vector.tensor_tensor(
                    t2v_t[:], sn_all[:, 2048 + mb:2048 + mb + 256], rs1[:],
                    op=MUL)
                w_t = w_pool.tile([128, 256], f32r, tag="w_t")
                nc.scalar.activation(
                    w_t[:], t2v_t[:], EXP, bias=bias_sb[:, m:m + 1], scale=TAU)
                xw_t = w_pool.tile([128, 256], f32r, tag="xw_t")
                if xe_pool_eng:
                    nc.gpsimd.tensor_tensor(
                        xw_t[:], t2v_t[:], w_t[:].bitcast(f32), op=MUL)
                else:
                    nc.vector.tensor_tensor(
                        xw_t[:], t2v_t[:], w_t[:].bitcast(f32), op=MUL)
                nc.tensor.matmul(
                    den3[:], sel_sb[:, 32 * m:32 * (m + 1)], w_t[:],
                    start=(m == 0), stop=(m == N_MT - 1),
                )
                nc.tensor.matmul(
                    num3[:], sel_sb[:, 32 * m:32 * (m + 1)], xw_t[:],
                    start=(m == 0), stop=(m == N_MT - 1),
                )
            den3_sb = t2v_pool.tile([32, 256], f32, tag="den3_sb")
            nc.scalar.copy(den3_sb[:], den3[:])
            rden3 = t2v_pool.tile([32, 256], f32, tag="rden3")
            nc.vector.reciprocal(rden3[:], den3_sb[:])
            t2v2 = t2v_pool.tile([32, 256], f32, tag="t2v2")
            # t2v2 = 0.5 * Num3 / Den3
            nc.vector.scalar_tensor_tensor(
                out=t2v2[:], in0=num3[:], scalar=0.5, in1=rden3[:],
                op0=MUL, op1=MUL,
            )
            out_sb = t2v_pool.tile([32, 256], f32, tag="out_sb")
            nc.vector.tensor_tensor(out_sb[:], t2v2[:], vt2_stage[:], op=ADD)
            nc.sync.dma_start(out=out_d.ap(), in_=out_sb[:])

    nc.compile()
    return nc


def _get_program(reps=1, xe_pool_eng=None, **_ignored):
    if xe_pool_eng is None:
        xe_pool_eng = os.environ.get("K_XE_POOL", "0") == "1"
    key = (reps, xe_pool_eng)
    if key not in _PROGRAM_CACHE:
        _PROGRAM_CACHE[key] = _build_program(reps, xe_pool_eng)
    return _PROGRAM_CACHE[key]


def _l2norm(a):
    n = np.linalg.norm(a, axis=-1, keepdims=True)
    return a / np.maximum(n, EPS)


def prepare_inputs(text_feat, video_feat, text_mask):
    """Host-side shard/layout prep. Returns in_maps for the 8 cores."""
    t = _l2norm(text_feat.astype(np.float32))          # [A, T, D]
    v = _l2norm(video_feat.astype(np.float32))         # [B, V, D]
    mask = text_mask.astype(np.float32)

    # video: [B, V, D] -> [D, B*V], shared by all cores
    vT = np.ascontiguousarray(v.reshape(B * V_FRM, D).T)

    p = np.arange(128)
    in_maps = []
    for c in range(N_CORES):
        tc_ = t[c * A_LOC:(c + 1) * A_LOC]             # [32, T, D]
        tT = np.ascontiguousarray(tc_.reshape(M_ROWS, D).T)   # [D, 1024]
        mk = mask[c * A_LOC:(c + 1) * A_LOC]           # [32, T]
        # selectors carry the 0/1 mask values: padded tokens contribute
        # exactly 0 to the partition-direction (over-t) sums
        sel = np.zeros((128, N_MT * 32), np.float32)
        sele = np.zeros((128, N_MT * 224), np.float32)
        for m in range(N_MT):
            mvals = mk[4 * m:4 * m + 4].reshape(128)   # mask for rows of tile m
            sel[p, m * 32 + 4 * m + p // 32] = mvals
            sele[p, m * 224 + 96 + 4 * m + p // 32] = mvals
        bias = np.full((128, N_MT + 1), SHIFT, np.float32)
        in_maps.append({"tT": tT, "vT": vT, "sel": sel, "sele": sele,
                        "bias": bias})
    return in_maps


def run(in_maps, trace=False, reps=1, **kwargs):
    import concourse.mybir as mybir
    from concourse import bass_utils

    nc = _get_program(reps=reps)
    # pad inputs to the program's declared shapes (bias width varies by build)
    shapes = {}
    for alloc in nc.m.functions[0].allocations:
        if isinstance(alloc, mybir.MemoryLocationSet) and alloc.kind == "ExternalInput":
            shapes[alloc.memorylocations[0].name] = tuple(alloc.tensor_shape)
    fixed = []
    for m in in_maps:
        mm = {}
        for k, v in m.items():
            shp = shapes.get(k, tuple(v.shape))
            if tuple(v.shape) != shp:
                out = np.full(shp, SHIFT if k == "bias" else 0.0, v.dtype)
                sl = tuple(slice(0, min(s, t)) for s, t in zip(v.shape, shp))
                out[sl] = v[sl]
                mm[k] = out
            else:
                mm[k] = v
        fixed.append(mm)
    return bass_utils.run_bass_kernel_spmd(
        nc, fixed, core_ids=list(range(N_CORES)), trace=trace, **kwargs
    )


def kernel(text_feat, video_feat, text_mask):
    in_maps = prepare_inputs(
        np.asarray(text_feat), np.asarray(video_feat), np.asarray(text_mask)
    )
    res = run(in_maps)
    out = np.concatenate([res.results[c]["out"] for c in range(N_CORES)], axis=0)
    return out.astype(np.float32)

